# revision 14
# baseline (speedup 1.0000x reference)
"""BoW extractor (VQ codebook softmax + border-cropped mean pool) on 8 Trainium2 cores.

Data-parallel over the batch dim: each of the 8 NeuronCores handles 16 images.
Per core, tokens are flattened to [3136, 768] (padded to 3200 = 25 tiles of 128)
and processed tile-by-tile:
  logits = 30 * (x @ emb.T) / ||x||   (fp32r matmuls, C contracted in 6 chunks)
  codes  = exp(logits) / row_sum      (ACT exp with per-partition scale + fused
                                       row-sums, DVE reciprocal + scale)
  bow    = L1norm(mean of kept codes) (PE matmul against a precomputed selector
                                       W, accumulated in one PSUM bank)
"""
import sys

sys.path.insert(0, "/opt/trn_rl_repo")

import numpy as np

N_CORES = 8
N_IMG = 128
C = 768
K = 4096
L = 196  # tokens per image after dropping CLS
IMG_PER_CORE = N_IMG // N_CORES  # 16
T_TOK = IMG_PER_CORE * L  # 3136
NT = 25  # 128-token tiles per core
T_PAD = NT * 128  # 3200
GRID = 14
SKIP = 2
INV_DELTA = 30.0  # 15.0 / 0.5
NORMALIZE_EPS = 1e-5

PROFILE = False
LAST_EXEC_NS = None

_PROG = None
_HOST_CONST = None


def _build_program():
    import concourse.bacc as bacc
    import concourse.tile as tile
    from concourse import mybir

    f32 = mybir.dt.float32
    f32r = mybir.dt.float32r
    bf16 = mybir.dt.bfloat16
    AF = mybir.ActivationFunctionType
    AX = mybir.AxisListType

    nc = bacc.Bacc("TRN2", target_bir_lowering=False, debug=False,
                   num_devices=N_CORES)
    xT_d = nc.dram_tensor("xT", [NT, 128, 6, 128], f32r, kind="ExternalInput")
    xn_d = nc.dram_tensor("xnat", [NT, 128, C], f32, kind="ExternalInput")
    emb_d = nc.dram_tensor("embT", [8, 128, 6, 512], f32r, kind="ExternalInput")
    w_d = nc.dram_tensor("W", [128, NT, 16], f32r, kind="ExternalInput")
    codes_d = nc.dram_tensor("codes", [NT, 128, 8, 512], f32r,
                             kind="ExternalOutput")
    bow_d = nc.dram_tensor("bow", [16, 8, 512], f32, kind="ExternalOutput")

    with tile.TileContext(nc) as tc:
        with (
            tc.tile_pool(name="const", bufs=1) as constp,
            tc.tile_pool(name="xT", bufs=3) as xTp,
            tc.tile_pool(name="xn", bufs=3) as xnp_,
            tc.tile_pool(name="sq", bufs=2) as sqp,
            tc.tile_pool(name="expp", bufs=2) as expp,
            tc.tile_pool(name="small", bufs=4) as smallp,
            tc.tile_pool(name="ps", bufs=5, space="PSUM") as psp,
            tc.tile_pool(name="psbow", bufs=2, space="PSUM") as psbowp,
            tc.tile_pool(name="pswarm", bufs=1, space="PSUM") as pswarmp,
        ):
            # PE warmup: dependency-free dummy matmuls run during the
            # initial DMAs so the HAM clock-gate opens (1.2->2.4 GHz)
            # before the real matmuls start.
            warm_sb = constp.tile([128, 128], bf16)
            nc.vector.memset(warm_sb[:], 0.0)
            warm_ps = pswarmp.tile([128, 128], f32)
            for _ in range(60):
                nc.tensor.matmul(warm_ps[:], warm_sb[:], warm_sb[:],
                                 start=True, stop=True)

            # codebook: blocks 0-1 on the ACT HWDGE ring, 2-7 via SWDGE
            # queues so the load runs in parallel and unblocks tile 0 fast
            emb_sb = constp.tile([128, 8, 6, 512], f32r)
            for j in range(8):
                eng = nc.scalar if j < 2 else nc.gpsimd
                eng.dma_start(out=emb_sb[:, j], in_=emb_d.ap()[j])
            w_sb = constp.tile([128, NT, 16], f32r)
            nc.scalar.dma_start(out=w_sb[:], in_=w_d.ap())
            # bow accumulator in SBUF: row i = image i, [8, 512] = code
            # blocks. Per-tile [16, 512] matmul results (PSUM) are
            # DVE-accumulated here.
            bow_acc = constp.tile([16, 8, 512], f32)
            nc.vector.memset(bow_acc[:], 0.0)

            for t in range(NT):
                xT_t = xTp.tile([128, 6, 128], f32r)
                nc.sync.dma_start(out=xT_t[:], in_=xT_d.ap()[t])
                xn_t = xnp_.tile([128, C], f32)
                nc.sync.dma_start(out=xn_t[:], in_=xn_d.ap()[t])

                # ||x||^2 per token via fused square + row-sum
                sq_t = sqp.tile([128, C], f32)
                n2 = smallp.tile([128, 1], f32)
                nc.scalar.activation(sq_t[:], xn_t[:], AF.Square,
                                     accum_out=n2[:])
                # sinv = 30/max(||x||, eps) = (max(n2, eps^2)/900)^-0.5,
                # computed as exp(-0.5 * ln(.)) — ln and exp live in the
                # same ACT table set as square, so no table reloads.
                nc.vector.tensor_scalar_max(n2[:], n2[:],
                                            NORMALIZE_EPS * NORMALIZE_EPS)
                lnn = smallp.tile([128, 1], f32)
                nc.scalar.activation(lnn[:], n2[:], AF.Ln,
                                     scale=1.0 / (INV_DELTA * INV_DELTA))
                sinv = smallp.tile([128, 1], f32)
                nc.scalar.activation(sinv[:], lnn[:], AF.Exp, scale=-0.5)

                sums = smallp.tile([128, 8], f32)
                exp_t = expp.tile([128, 8, 512], f32r)
                for j in range(8):
                    ps = psp.tile([128, 512], f32)
                    for c in range(6):
                        nc.tensor.matmul(
                            ps[:],
                            xT_t[:, c, :],
                            emb_sb[:, j, c, :],
                            start=(c == 0),
                            stop=(c == 5),
                        )
                    nc.scalar.activation(exp_t[:, j, :], ps[:], AF.Exp,
                                         scale=sinv[:],
                                         accum_out=sums[:, j:j + 1])

                denom = smallp.tile([128, 1], f32)
                nc.vector.reduce_sum(denom[:], sums[:], axis=AX.X)
                r = smallp.tile([128, 1], f32)
                nc.vector.reciprocal(r[:], denom[:])

                for j in range(8):
                    nc.vector.tensor_scalar_mul(exp_t[:, j, :],
                                                exp_t[:, j, :], r[:])
                    bow_tmp = psbowp.tile([16, 512], f32)
                    nc.tensor.matmul(bow_tmp[:], w_sb[:, t, :],
                                     exp_t[:, j, :], start=True, stop=True)
                    nc.vector.tensor_add(bow_acc[:, j, :], bow_acc[:, j, :],
                                         bow_tmp[:])
                nc.gpsimd.dma_start(out=codes_d.ap()[t], in_=exp_t[:])

            # L1-normalize bow per image (rows are images already)
            ssum = smallp.tile([16, 1], f32)
            nc.vector.reduce_sum(ssum[:], bow_acc[:], axis=AX.XY)
            nc.vector.tensor_scalar_max(ssum[:], ssum[:], NORMALIZE_EPS)
            rimg = smallp.tile([16, 1], f32)
            nc.vector.reciprocal(rimg[:], ssum[:])
            nc.vector.tensor_scalar_mul(bow_acc[:], bow_acc[:], rimg[:])
            nc.gpsimd.dma_start(out=bow_d.ap(), in_=bow_acc[:])

    nc.compile()
    return nc


def _host_constants():
    global _HOST_CONST
    if _HOST_CONST is not None:
        return _HOST_CONST
    # kept-token mask on the 14x14 grid (drop SKIP-wide border)
    l_idx = np.arange(L)
    row, col = l_idx // GRID, l_idx % GRID
    kept = ((row >= SKIP) & (row < GRID - SKIP) &
            (col >= SKIP) & (col < GRID - SKIP))
    n_keep = int(kept.sum())  # 100
    w_full = np.zeros((T_PAD, IMG_PER_CORE), np.float32)
    for i in range(IMG_PER_CORE):
        w_full[i * L:(i + 1) * L, i] = kept / float(n_keep)
    w3 = np.ascontiguousarray(
        w_full.reshape(NT, 128, IMG_PER_CORE).transpose(1, 0, 2))
    _HOST_CONST = w3
    return _HOST_CONST


def _get_program():
    global _PROG
    if _PROG is None:
        _PROG = _build_program()
    return _PROG


def kernel(x, embedding):
    global LAST_EXEC_NS
    from concourse.bass_utils import run_bass_kernel_spmd

    x = np.asarray(x, dtype=np.float32)
    embedding = np.asarray(embedding, dtype=np.float32)
    nc = _get_program()
    w3 = _host_constants()

    embT = np.ascontiguousarray(
        embedding.T.reshape(6, 128, 8, 512).transpose(2, 1, 0, 3))

    in_maps = []
    for core in range(N_CORES):
        xc = x[core * IMG_PER_CORE:(core + 1) * IMG_PER_CORE, 1:, :]
        xp = np.zeros((T_PAD, C), np.float32)
        xp[:T_TOK] = xc.reshape(T_TOK, C)
        xp[T_TOK:, 0] = 1.0  # pad tokens: unit norm, zero pool weight
        in_maps.append({
            "xT": np.ascontiguousarray(
                xp.reshape(NT, 128, 6, 128).transpose(0, 3, 2, 1)),
            "xnat": xp.reshape(NT, 128, C),
            "embT": embT,
            "W": w3,
        })

    res = run_bass_kernel_spmd(nc, in_maps, core_ids=list(range(N_CORES)),
                               trace=PROFILE)
    LAST_EXEC_NS = res.exec_time_ns

    bow = np.empty((N_IMG, K), np.float32)
    codes = np.empty((N_IMG, L, K), np.float32)
    for core in range(N_CORES):
        sl = slice(core * IMG_PER_CORE, (core + 1) * IMG_PER_CORE)
        codes[sl] = (res.results[core]["codes"]
                     .reshape(T_PAD, K)[:T_TOK]
                     .reshape(IMG_PER_CORE, L, K))
        bow[sl] = res.results[core]["bow"].reshape(IMG_PER_CORE, K)
    return bow, codes


# revision 17
# speedup vs baseline: 1.0466x; 1.0466x over previous
"""BoW extractor (VQ codebook softmax + border-cropped mean pool) on 8 Trainium2 cores.

Data-parallel over the batch dim: each of the 8 NeuronCores handles 16 images.
Per core, tokens are flattened to [3136, 768] (padded to 3200 = 25 tiles of 128)
and processed tile-by-tile:
  logits = 30 * (x @ emb.T) / ||x||   (fp32r matmuls, C contracted in 6 chunks)
  codes  = exp(logits) / row_sum      (ACT exp with per-partition scale + fused
                                       row-sums, DVE reciprocal + scale)
  bow    = L1norm(mean of kept codes) (PE matmul against a precomputed selector
                                       W, accumulated in one PSUM bank)
"""
import sys

sys.path.insert(0, "/opt/trn_rl_repo")

import numpy as np

N_CORES = 8
N_IMG = 128
C = 768
K = 4096
L = 196  # tokens per image after dropping CLS
IMG_PER_CORE = N_IMG // N_CORES  # 16
T_TOK = IMG_PER_CORE * L  # 3136
NT = 25  # 128-token tiles per core
T_PAD = NT * 128  # 3200
GRID = 14
SKIP = 2
INV_DELTA = 30.0  # 15.0 / 0.5
NORMALIZE_EPS = 1e-5

PROFILE = False
LAST_EXEC_NS = None

_PROG = None
_HOST_CONST = None


def _build_program():
    import concourse.bacc as bacc
    import concourse.tile as tile
    from concourse import mybir

    f32 = mybir.dt.float32
    f32r = mybir.dt.float32r
    bf16 = mybir.dt.bfloat16
    AF = mybir.ActivationFunctionType
    AX = mybir.AxisListType

    nc = bacc.Bacc("TRN2", target_bir_lowering=False, debug=False,
                   num_devices=N_CORES)
    xT_d = nc.dram_tensor("xT", [NT, 128, 6, 128], f32r, kind="ExternalInput")
    xn_d = nc.dram_tensor("xnat", [NT, 128, C], f32, kind="ExternalInput")
    emb_d = nc.dram_tensor("embT", [8, 128, 6, 512], f32r, kind="ExternalInput")
    w_d = nc.dram_tensor("W", [128, NT, 16], f32r, kind="ExternalInput")
    codes_d = nc.dram_tensor("codes", [NT, 128, 8, 512], f32r,
                             kind="ExternalOutput")
    bow_d = nc.dram_tensor("bow", [16, 8, 512], f32, kind="ExternalOutput")

    with tile.TileContext(nc) as tc:
        with (
            tc.tile_pool(name="const", bufs=1) as constp,
            tc.tile_pool(name="xT", bufs=3) as xTp,
            tc.tile_pool(name="xn", bufs=3) as xnp_,
            tc.tile_pool(name="sq", bufs=2) as sqp,
            tc.tile_pool(name="expp", bufs=2) as expp,
            tc.tile_pool(name="small", bufs=4) as smallp,
            tc.tile_pool(name="ps", bufs=5, space="PSUM") as psp,
            tc.tile_pool(name="psbow", bufs=2, space="PSUM") as psbowp,
            tc.tile_pool(name="pswarm", bufs=1, space="PSUM") as pswarmp,
        ):
            # PE warmup: dependency-free dummy matmuls run during the
            # initial DMAs so the HAM clock-gate opens (1.2->2.4 GHz)
            # before the real matmuls start.
            warm_sb = constp.tile([128, 128], bf16)
            nc.vector.memset(warm_sb[:], 0.0)
            warm_ps = pswarmp.tile([128, 128], f32)
            for _ in range(60):
                nc.tensor.matmul(warm_ps[:], warm_sb[:], warm_sb[:],
                                 start=True, stop=True)

            # load the one ACT table set that covers square, ln and exp so
            # walrus never inserts per-tile table reloads
            with tc.tile_critical():
                nc.scalar.add_instruction(mybir.InstLoadActFuncSet(
                    name=nc.get_next_instruction_name(),
                    act_func_set_id=6,  # natural_log_exp_and_others,
                    ins=[], outs=[]))

            # prefetch the first 3 token tiles ahead of the codebook on the
            # SP ring; codebook blocks 0-3 ride the ACT ring, 4-7 the SP
            # ring, so the full codebook lands in ~half the serial time
            pre_xT, pre_xn = [], []
            for t in range(3):
                a = xTp.tile([128, 6, 128], f32r, name=f"xTpre{t}", tag="xT_t")
                nc.sync.dma_start(out=a[:], in_=xT_d.ap()[t])
                pre_xT.append(a)
                bpre = xnp_.tile([128, C], f32, name=f"xnpre{t}", tag="xn_t")
                nc.sync.dma_start(out=bpre[:], in_=xn_d.ap()[t])
                pre_xn.append(bpre)
            emb_sb = constp.tile([128, 8, 6, 512], f32r)
            for j in range(8):
                eng = nc.scalar if j < 4 else nc.sync
                eng.dma_start(out=emb_sb[:, j], in_=emb_d.ap()[j])
            w_sb = constp.tile([128, NT, 16], f32r)
            nc.scalar.dma_start(out=w_sb[:], in_=w_d.ap())
            # bow accumulator in SBUF: row i = image i, [8, 512] = code
            # blocks. Per-tile [16, 512] matmul results (PSUM) are
            # DVE-accumulated here.
            bow_acc = constp.tile([16, 8, 512], f32)
            nc.gpsimd.memset(bow_acc[:], 0.0)

            for t in range(NT):
                if t < 3:
                    xT_t, xn_t = pre_xT[t], pre_xn[t]
                else:
                    xT_t = xTp.tile([128, 6, 128], f32r)
                    nc.sync.dma_start(out=xT_t[:], in_=xT_d.ap()[t])
                    xn_t = xnp_.tile([128, C], f32)
                    nc.sync.dma_start(out=xn_t[:], in_=xn_d.ap()[t])

                # ||x||^2 per token via fused square + row-sum
                sq_t = sqp.tile([128, C], f32)
                n2 = smallp.tile([128, 1], f32)
                nc.scalar.activation(sq_t[:], xn_t[:], AF.Square,
                                     accum_out=n2[:])
                # sinv = 30/max(||x||, eps) = (max(n2, eps^2)/900)^-0.5,
                # computed as exp(-0.5 * ln(.)) — ln and exp live in the
                # same ACT table set as square, so no table reloads.
                nc.vector.tensor_scalar_max(n2[:], n2[:],
                                            NORMALIZE_EPS * NORMALIZE_EPS)
                lnn = smallp.tile([128, 1], f32)
                nc.scalar.activation(lnn[:], n2[:], AF.Ln,
                                     scale=1.0 / (INV_DELTA * INV_DELTA))
                sinv = smallp.tile([128, 1], f32)
                nc.scalar.activation(sinv[:], lnn[:], AF.Exp, scale=-0.5)

                sums = smallp.tile([128, 8], f32)
                exp_t = expp.tile([128, 8, 512], f32r)
                for j in range(8):
                    ps = psp.tile([128, 512], f32)
                    for c in range(6):
                        nc.tensor.matmul(
                            ps[:],
                            xT_t[:, c, :],
                            emb_sb[:, j, c, :],
                            start=(c == 0),
                            stop=(c == 5),
                        )
                    nc.scalar.activation(exp_t[:, j, :], ps[:], AF.Exp,
                                         scale=sinv[:],
                                         accum_out=sums[:, j:j + 1])

                denom = smallp.tile([128, 1], f32)
                nc.vector.reduce_sum(denom[:], sums[:], axis=AX.X)
                r = smallp.tile([128, 1], f32)
                nc.vector.reciprocal(r[:], denom[:])

                for j in range(8):
                    nc.vector.tensor_scalar_mul(exp_t[:, j, :],
                                                exp_t[:, j, :], r[:])
                    bow_tmp = psbowp.tile([16, 512], f32)
                    nc.tensor.matmul(bow_tmp[:], w_sb[:, t, :],
                                     exp_t[:, j, :], start=True, stop=True)
                    nc.vector.tensor_add(bow_acc[:, j, :], bow_acc[:, j, :],
                                         bow_tmp[:])
                nc.gpsimd.dma_start(out=codes_d.ap()[t], in_=exp_t[:])

            # L1-normalize bow per image (rows are images already)
            ssum = smallp.tile([16, 1], f32)
            nc.vector.reduce_sum(ssum[:], bow_acc[:], axis=AX.XY)
            nc.vector.tensor_scalar_max(ssum[:], ssum[:], NORMALIZE_EPS)
            rimg = smallp.tile([16, 1], f32)
            nc.vector.reciprocal(rimg[:], ssum[:])
            nc.vector.tensor_scalar_mul(bow_acc[:], bow_acc[:], rimg[:])
            nc.gpsimd.dma_start(out=bow_d.ap(), in_=bow_acc[:])

    nc.compile()
    return nc


def _host_constants():
    global _HOST_CONST
    if _HOST_CONST is not None:
        return _HOST_CONST
    # kept-token mask on the 14x14 grid (drop SKIP-wide border)
    l_idx = np.arange(L)
    row, col = l_idx // GRID, l_idx % GRID
    kept = ((row >= SKIP) & (row < GRID - SKIP) &
            (col >= SKIP) & (col < GRID - SKIP))
    n_keep = int(kept.sum())  # 100
    w_full = np.zeros((T_PAD, IMG_PER_CORE), np.float32)
    for i in range(IMG_PER_CORE):
        w_full[i * L:(i + 1) * L, i] = kept / float(n_keep)
    w3 = np.ascontiguousarray(
        w_full.reshape(NT, 128, IMG_PER_CORE).transpose(1, 0, 2))
    _HOST_CONST = w3
    return _HOST_CONST


def _get_program():
    global _PROG
    if _PROG is None:
        _PROG = _build_program()
    return _PROG


def kernel(x, embedding):
    global LAST_EXEC_NS
    from concourse.bass_utils import run_bass_kernel_spmd

    x = np.asarray(x, dtype=np.float32)
    embedding = np.asarray(embedding, dtype=np.float32)
    nc = _get_program()
    w3 = _host_constants()

    embT = np.ascontiguousarray(
        embedding.T.reshape(6, 128, 8, 512).transpose(2, 1, 0, 3))

    in_maps = []
    for core in range(N_CORES):
        xc = x[core * IMG_PER_CORE:(core + 1) * IMG_PER_CORE, 1:, :]
        xp = np.zeros((T_PAD, C), np.float32)
        xp[:T_TOK] = xc.reshape(T_TOK, C)
        xp[T_TOK:, 0] = 1.0  # pad tokens: unit norm, zero pool weight
        in_maps.append({
            "xT": np.ascontiguousarray(
                xp.reshape(NT, 128, 6, 128).transpose(0, 3, 2, 1)),
            "xnat": xp.reshape(NT, 128, C),
            "embT": embT,
            "W": w3,
        })

    res = run_bass_kernel_spmd(nc, in_maps, core_ids=list(range(N_CORES)),
                               trace=PROFILE)
    LAST_EXEC_NS = res.exec_time_ns

    bow = np.empty((N_IMG, K), np.float32)
    codes = np.empty((N_IMG, L, K), np.float32)
    for core in range(N_CORES):
        sl = slice(core * IMG_PER_CORE, (core + 1) * IMG_PER_CORE)
        codes[sl] = (res.results[core]["codes"]
                     .reshape(T_PAD, K)[:T_TOK]
                     .reshape(IMG_PER_CORE, L, K))
        bow[sl] = res.results[core]["bow"].reshape(IMG_PER_CORE, K)
    return bow, codes


# revision 18
# speedup vs baseline: 1.0558x; 1.0088x over previous
"""BoW extractor (VQ codebook softmax + border-cropped mean pool) on 8 Trainium2 cores.

Data-parallel over the batch dim: each of the 8 NeuronCores handles 16 images.
Per core, tokens are flattened to [3136, 768] (padded to 3200 = 25 tiles of 128)
and processed tile-by-tile:
  logits = 30 * (x @ emb.T) / ||x||   (fp32r matmuls, C contracted in 6 chunks)
  codes  = exp(logits) / row_sum      (ACT exp with per-partition scale + fused
                                       row-sums, DVE reciprocal + scale)
  bow    = L1norm(mean of kept codes) (PE matmul against a precomputed selector
                                       W, accumulated in one PSUM bank)
"""
import sys

sys.path.insert(0, "/opt/trn_rl_repo")

import numpy as np

N_CORES = 8
N_IMG = 128
C = 768
K = 4096
L = 196  # tokens per image after dropping CLS
IMG_PER_CORE = N_IMG // N_CORES  # 16
T_TOK = IMG_PER_CORE * L  # 3136
NT = 25  # 128-token tiles per core
T_PAD = NT * 128  # 3200
GRID = 14
SKIP = 2
INV_DELTA = 30.0  # 15.0 / 0.5
NORMALIZE_EPS = 1e-5

PROFILE = False
LAST_EXEC_NS = None

_PROG = None
_HOST_CONST = None


def _build_program():
    import concourse.bacc as bacc
    import concourse.tile as tile
    from concourse import mybir

    f32 = mybir.dt.float32
    f32r = mybir.dt.float32r
    bf16 = mybir.dt.bfloat16
    AF = mybir.ActivationFunctionType
    AX = mybir.AxisListType

    nc = bacc.Bacc("TRN2", target_bir_lowering=False, debug=False,
                   num_devices=N_CORES)
    xT_d = nc.dram_tensor("xT", [NT, 128, 6, 128], f32r, kind="ExternalInput")
    xn_d = nc.dram_tensor("xnat", [NT, 128, C], f32, kind="ExternalInput")
    emb_d = nc.dram_tensor("embT", [8, 128, 6, 512], f32r, kind="ExternalInput")
    w_d = nc.dram_tensor("W", [128, NT, 16], f32r, kind="ExternalInput")
    codes_d = nc.dram_tensor("codes", [NT, 128, 8, 512], f32r,
                             kind="ExternalOutput")
    bow_d = nc.dram_tensor("bow", [16, 8, 512], f32, kind="ExternalOutput")

    with tile.TileContext(nc) as tc:
        with (
            tc.tile_pool(name="const", bufs=1) as constp,
            tc.tile_pool(name="xT", bufs=3) as xTp,
            tc.tile_pool(name="xn", bufs=3) as xnp_,
            tc.tile_pool(name="sq", bufs=2) as sqp,
            tc.tile_pool(name="expp", bufs=2) as expp,
            tc.tile_pool(name="small", bufs=4) as smallp,
            tc.tile_pool(name="ps", bufs=5, space="PSUM") as psp,
            tc.tile_pool(name="psbow", bufs=2, space="PSUM") as psbowp,
            tc.tile_pool(name="pswarm", bufs=1, space="PSUM") as pswarmp,
        ):
            # PE warmup: dependency-free dummy matmuls run during the
            # initial DMAs so the HAM clock-gate opens (1.2->2.4 GHz)
            # before the real matmuls start.
            warm_sb = constp.tile([128, 128], bf16)
            nc.vector.memset(warm_sb[:], 0.0)
            warm_ps = pswarmp.tile([128, 128], f32)
            for _ in range(60):
                nc.tensor.matmul(warm_ps[:], warm_sb[:], warm_sb[:],
                                 start=True, stop=True)

            # load the one ACT table set that covers square, ln and exp so
            # walrus never inserts per-tile table reloads
            with tc.tile_critical():
                nc.scalar.add_instruction(mybir.InstLoadActFuncSet(
                    name=nc.get_next_instruction_name(),
                    act_func_set_id=6,  # natural_log_exp_and_others,
                    ins=[], outs=[]))

            # prefetch the first 3 token tiles ahead of the codebook on the
            # SP ring; codebook blocks 0-3 ride the ACT ring, 4-7 the SP
            # ring, so the full codebook lands in ~half the serial time
            pre_xT, pre_xn = [], []
            for t in range(3):
                a = xTp.tile([128, 6, 128], f32r, name=f"xTpre{t}", tag="xT_t")
                nc.sync.dma_start(out=a[:], in_=xT_d.ap()[t])
                pre_xT.append(a)
                bpre = xnp_.tile([128, C], f32, name=f"xnpre{t}", tag="xn_t")
                nc.sync.dma_start(out=bpre[:], in_=xn_d.ap()[t])
                pre_xn.append(bpre)
            emb_sb = constp.tile([128, 8, 6, 512], f32r)
            for j in range(8):
                eng = nc.scalar if j < 4 else nc.sync
                eng.dma_start(out=emb_sb[:, j], in_=emb_d.ap()[j])
            w_sb = constp.tile([128, NT, 16], f32r)
            nc.scalar.dma_start(out=w_sb[:], in_=w_d.ap())
            # bow accumulator in SBUF: row i = image i, [8, 512] = code
            # blocks. Per-tile [16, 512] matmul results (PSUM) are
            # DVE-accumulated here.
            bow_acc = constp.tile([16, 8, 512], f32)
            nc.gpsimd.memset(bow_acc[:], 0.0)

            # Norm pipeline, one 5-tile batch ahead of the main loop: the
            # ACT sqrt runs once per batch instead of per tile, so the two
            # table-set reloads (sqrt<->exp) cost 10 loads total, not 50.
            n2_all = constp.tile([128, NT], f32)
            sinv_all = constp.tile([128, NT], f32)
            NB = 5

            def norm_batch(g):
                lo, hi = NB * g, min(NB * (g + 1), NT)
                for u in range(lo, hi):
                    if u < 3:
                        xn_u = pre_xn[u]
                    else:
                        xn_u = xnp_.tile([128, C], f32, name=f"xn{u}",
                                         tag="xn_t")
                        nc.sync.dma_start(out=xn_u[:], in_=xn_d.ap()[u])
                    sq_u = sqp.tile([128, C], f32, name=f"sq{u}", tag="sq_t")
                    nc.scalar.activation(sq_u[:], xn_u[:], AF.Square,
                                         accum_out=n2_all[:, u:u + 1])
                nc.vector.tensor_scalar_max(n2_all[:, lo:hi],
                                            n2_all[:, lo:hi],
                                            NORMALIZE_EPS * NORMALIZE_EPS)
                nrm = smallp.tile([128, NB], f32, name=f"nrm{g}", tag="nrm")
                nc.scalar.activation(nrm[:, :hi - lo], n2_all[:, lo:hi],
                                     AF.Sqrt,
                                     scale=1.0 / (INV_DELTA * INV_DELTA))
                nc.vector.reciprocal(sinv_all[:, lo:hi], nrm[:, :hi - lo])

            norm_batch(0)

            for t in range(NT):
                if t % NB == 0 and t + NB < NT + NB:
                    if t // NB + 1 <= (NT - 1) // NB:
                        norm_batch(t // NB + 1)
                if t < 3:
                    xT_t = pre_xT[t]
                else:
                    xT_t = xTp.tile([128, 6, 128], f32r)
                    nc.sync.dma_start(out=xT_t[:], in_=xT_d.ap()[t])
                sinv = sinv_all[:, t:t + 1]

                sums = smallp.tile([128, 8], f32)
                exp_t = expp.tile([128, 8, 512], f32r)
                for j in range(8):
                    ps = psp.tile([128, 512], f32)
                    for c in range(6):
                        nc.tensor.matmul(
                            ps[:],
                            xT_t[:, c, :],
                            emb_sb[:, j, c, :],
                            start=(c == 0),
                            stop=(c == 5),
                        )
                    nc.scalar.activation(exp_t[:, j, :], ps[:], AF.Exp,
                                         scale=sinv[:],
                                         accum_out=sums[:, j:j + 1])

                denom = smallp.tile([128, 1], f32)
                nc.vector.reduce_sum(denom[:], sums[:], axis=AX.X)
                r = smallp.tile([128, 1], f32)
                nc.vector.reciprocal(r[:], denom[:])

                for j in range(8):
                    nc.vector.tensor_scalar_mul(exp_t[:, j, :],
                                                exp_t[:, j, :], r[:])
                    bow_tmp = psbowp.tile([16, 512], f32)
                    nc.tensor.matmul(bow_tmp[:], w_sb[:, t, :],
                                     exp_t[:, j, :], start=True, stop=True)
                    nc.vector.tensor_add(bow_acc[:, j, :], bow_acc[:, j, :],
                                         bow_tmp[:])
                nc.gpsimd.dma_start(out=codes_d.ap()[t], in_=exp_t[:])

            # L1-normalize bow per image (rows are images already)
            ssum = smallp.tile([16, 1], f32)
            nc.vector.reduce_sum(ssum[:], bow_acc[:], axis=AX.XY)
            nc.vector.tensor_scalar_max(ssum[:], ssum[:], NORMALIZE_EPS)
            rimg = smallp.tile([16, 1], f32)
            nc.vector.reciprocal(rimg[:], ssum[:])
            nc.vector.tensor_scalar_mul(bow_acc[:], bow_acc[:], rimg[:])
            nc.gpsimd.dma_start(out=bow_d.ap(), in_=bow_acc[:])

    nc.compile()
    return nc


def _host_constants():
    global _HOST_CONST
    if _HOST_CONST is not None:
        return _HOST_CONST
    # kept-token mask on the 14x14 grid (drop SKIP-wide border)
    l_idx = np.arange(L)
    row, col = l_idx // GRID, l_idx % GRID
    kept = ((row >= SKIP) & (row < GRID - SKIP) &
            (col >= SKIP) & (col < GRID - SKIP))
    n_keep = int(kept.sum())  # 100
    w_full = np.zeros((T_PAD, IMG_PER_CORE), np.float32)
    for i in range(IMG_PER_CORE):
        w_full[i * L:(i + 1) * L, i] = kept / float(n_keep)
    w3 = np.ascontiguousarray(
        w_full.reshape(NT, 128, IMG_PER_CORE).transpose(1, 0, 2))
    _HOST_CONST = w3
    return _HOST_CONST


def _get_program():
    global _PROG
    if _PROG is None:
        _PROG = _build_program()
    return _PROG


def kernel(x, embedding):
    global LAST_EXEC_NS
    from concourse.bass_utils import run_bass_kernel_spmd

    x = np.asarray(x, dtype=np.float32)
    embedding = np.asarray(embedding, dtype=np.float32)
    nc = _get_program()
    w3 = _host_constants()

    embT = np.ascontiguousarray(
        embedding.T.reshape(6, 128, 8, 512).transpose(2, 1, 0, 3))

    in_maps = []
    for core in range(N_CORES):
        xc = x[core * IMG_PER_CORE:(core + 1) * IMG_PER_CORE, 1:, :]
        xp = np.zeros((T_PAD, C), np.float32)
        xp[:T_TOK] = xc.reshape(T_TOK, C)
        xp[T_TOK:, 0] = 1.0  # pad tokens: unit norm, zero pool weight
        in_maps.append({
            "xT": np.ascontiguousarray(
                xp.reshape(NT, 128, 6, 128).transpose(0, 3, 2, 1)),
            "xnat": xp.reshape(NT, 128, C),
            "embT": embT,
            "W": w3,
        })

    res = run_bass_kernel_spmd(nc, in_maps, core_ids=list(range(N_CORES)),
                               trace=PROFILE)
    LAST_EXEC_NS = res.exec_time_ns

    bow = np.empty((N_IMG, K), np.float32)
    codes = np.empty((N_IMG, L, K), np.float32)
    for core in range(N_CORES):
        sl = slice(core * IMG_PER_CORE, (core + 1) * IMG_PER_CORE)
        codes[sl] = (res.results[core]["codes"]
                     .reshape(T_PAD, K)[:T_TOK]
                     .reshape(IMG_PER_CORE, L, K))
        bow[sl] = res.results[core]["bow"].reshape(IMG_PER_CORE, K)
    return bow, codes
